# revision 34
# baseline (speedup 1.0000x reference)
"""DiceLoss kernel for Trainium2, data-parallel over 8 NeuronCores.

Algorithm (per core, 2 of 16 batches):
  - argmax one-hot lanes E = (e0, e1, e2, 1) with e_c = [x_c == max_c(x)],
    computed exactly: e0/e1 as f32 is_equal on the Vector engine, e2 via
    d2 = x2 - mx on Pool (exactly 0 iff x2 == mx) then d2 == 0.
  - target MOMENT lanes T = (1, t, t^2, |t-1|) straight from the uint8
    labels on the Scalar engine (one ACT op per lane; values are small
    ints, exact in bf16).
  - Both lane sets are written bf16, interleaved class-minor into
    [128, 4*fd] tiles; the TensorEngine accumulates
    O += E_chunk^T @ T_chunk over 128-wide chunks in PSUM. The 4x4
    diagonal blocks of O sum to M'[c, j] = sum_pix e_c * mu_j(t).
  - Host sums the 8 per-core [128,128] PSUM dumps, inverts the 4x4 moment
    basis (exact integers) to get the confusion matrix, and finishes the
    (2i+eps)/(u+eps) division and the mean in f32 like the reference.

All sums are integer-valued f32 < 2^24, so the result matches the jax
reference bit-for-bit (up to argmax ties that are bit-equal in f32).
"""
import sys

sys.path.insert(0, "/opt/trn_rl_repo")

import numpy as np

B, C, H, W = 16, 4, 512, 512
N_CORES = 8
B_LOC = B // N_CORES          # 2 batches per core
EPS = 1e-6
P = 128                       # SBUF partitions
FD = 1024                     # max free-dim of one pixel tile
PLANE = H * W                 # 262144 pixels per (b, c) plane

# Pixel segments per core: (batch, flat_start, fd). Each covers pixels
# [start, start + 128*fd) of that batch's plane; partition k owns
# [start + k*fd, start + (k+1)*fd). Trailing segments are smaller so the
# post-last-DMA compute tail is short.
SEGS = [
    (0, 0, 512),
    (0, 65536, 1024),
    (0, 196608, 512),
    (1, 0, 1024),
    (1, 131072, 768),
    (1, 229376, 256),
]
assert sum(128 * fd for b, s, fd in SEGS) == B_LOC * PLANE
NT = len(SEGS)
NCH_TOT = sum(4 * fd // 128 for _, _, fd in SEGS)


def build_body(tc, outs, ins, n_reps=1):
    """Kernel body. ins = {"x": AP [B_LOC,C,H,W] f32, "t": AP [B_LOC,H,W] u8}
    outs = {"conf": AP [128,128] f32}. n_reps>1 repeats the whole pass
    (PSUM keeps accumulating; used for timing-by-differencing)."""
    import concourse.mybir as mybir

    nc = tc.nc
    f32 = mybir.dt.float32
    bf16 = mybir.dt.bfloat16
    AF = mybir.ActivationFunctionType
    OP = mybir.AluOpType

    x = ins["x"]
    t = ins["t"]
    conf = outs["conf"]

    xf = x.rearrange("b c h w -> b c (h w)")
    tfl = t.rearrange("b h w -> b (h w)")

    NEB = 3  # E/T buffer count
    with (
        tc.tile_pool(name="xin", bufs=4) as xin,
        tc.tile_pool(name="work", bufs=3) as work,
        tc.tile_pool(name="eht", bufs=1) as eht,
        tc.tile_pool(name="psum", bufs=1, space="PSUM") as psum,
    ):
        P_acc = psum.tile([P, 128], f32, name="P_acc")
        bias_m1 = eht.tile([P, 1], f32, name="bias_m1")
        nc.gpsimd.memset(bias_m1, -1.0)
        Es = [eht.tile([P, FD * 4], bf16, name=f"Ebuf{i}") for i in range(NEB)]
        Ts = [eht.tile([P, FD * 4], bf16, name=f"Tbuf{i}") for i in range(NEB)]
        for buf in Es:
            b4 = buf.rearrange("p (f c) -> p f c", c=4)
            nc.gpsimd.memset(b4[:, :, 3], 1.0)
        for buf in Ts:
            b4 = buf.rearrange("p (f c) -> p f c", c=4)
            nc.gpsimd.memset(b4[:, :, 0], 1.0)

        n_mm = n_reps * NCH_TOT
        mm = 0
        for it_g in range(n_reps * NT):
            it = it_g % NT
            b_i, seg_start, fd = SEGS[it]
            npix = P * fd

            xts = [None] * C
            tu = None

            def dma_x(c):
                xc = xin.tile([P, FD], f32, name=f"xc{c}")[:, :fd]
                nc.sync.dma_start(
                    out=xc,
                    in_=xf[b_i, c, seg_start : seg_start + npix].rearrange(
                        "(p f) -> p f", f=fd
                    ),
                )
                xts[c] = xc

            dma_x(0)
            dma_x(1)
            tu = xin.tile([P, FD], mybir.dt.uint8, name="tu")[:, :fd]
            nc.sync.dma_start(
                out=tu,
                in_=tfl[b_i, seg_start : seg_start + npix].rearrange(
                    "(p f) -> p f", f=fd
                ),
            )
            dma_x(2)
            dma_x(3)

            # max over the 4 class planes (Pool has no TensorTensor max)
            m01 = work.tile([P, FD], f32, name="m01")[:, :fd]
            m23 = work.tile([P, FD], f32, name="m23")[:, :fd]
            mx = work.tile([P, FD], f32, name="mx")[:, :fd]
            nc.vector.tensor_tensor(m01, xts[0], xts[1], OP.max)
            nc.vector.tensor_tensor(m23, xts[2], xts[3], OP.max)
            nc.vector.tensor_tensor(mx, m01, m23, OP.max)

            E = Es[it_g % NEB]
            T = Ts[it_g % NEB]
            E4 = E[:, : 4 * fd].rearrange("p (f c) -> p f c", c=4)
            T4 = T[:, : 4 * fd].rearrange("p (f c) -> p f c", c=4)

            # pred one-hot lanes 0..2 (lane 3 stays 1.0):
            #   e0, e1 on DVE via is_equal(x_c, mx)
            #   e2 on Pool via d2 = x2 - mx (exactly 0 iff x2 == mx), then
            #   d2 == 0. On the last segment e2 runs on DVE: the Pool chain
            #   would sit on the critical tail.
            nc.vector.tensor_tensor(E4[:, :, 0], xts[0], mx, OP.is_equal)
            nc.vector.tensor_tensor(E4[:, :, 1], xts[1], mx, OP.is_equal)
            if it_g == n_reps * NT - 1:
                nc.vector.tensor_tensor(E4[:, :, 2], xts[2], mx, OP.is_equal)
            else:
                d2 = work.tile([P, FD], f32, name="d2")[:, :fd]
                nc.gpsimd.tensor_tensor(d2, xts[2], mx, OP.subtract)
                nc.gpsimd.tensor_scalar(E4[:, :, 2], d2, 0.0, None, OP.is_equal)

            # target MOMENT lanes on ACT, straight from the uint8 labels:
            #   lane 0 = 1 (memset), lane 1 = t, lane 2 = t^2, lane 3 = |t-1|
            # All values are small ints -> exact in bf16; the host inverts the
            # 4x4 moment basis to recover per-class counts.
            nc.scalar.copy(T4[:, :, 1], tu)
            nc.scalar.activation(T4[:, :, 2], tu, AF.Square)
            nc.scalar.activation(T4[:, :, 3], tu, AF.Abs, bias=bias_m1, scale=1.0)

            for w_i in range(4 * fd // 128):
                sl = slice(w_i * 128, (w_i + 1) * 128)
                nc.tensor.matmul(
                    P_acc,
                    E[:, sl],
                    T[:, sl],
                    start=(mm == 0),
                    stop=(mm == n_mm - 1),
                )
                mm += 1

        conf_sb = eht.tile([P, 128], f32, name="conf_sb")
        nc.scalar.copy(conf_sb, P_acc)
        nc.sync.dma_start(out=conf, in_=conf_sb)


_NC_CACHE = {}


def _get_nc(n_reps=1):
    if n_reps in _NC_CACHE:
        return _NC_CACHE[n_reps]
    import concourse.bacc as bacc
    import concourse.mybir as mybir
    import concourse.tile as tile

    nc = bacc.Bacc(
        "TRN2",
        target_bir_lowering=False,
        debug=False,
        enable_asserts=False,
        num_devices=N_CORES,
    )
    x = nc.dram_tensor("x", [B_LOC, C, H, W], mybir.dt.float32, kind="ExternalInput").ap()
    t = nc.dram_tensor("t", [B_LOC, H, W], mybir.dt.uint8, kind="ExternalInput").ap()
    conf = nc.dram_tensor("conf", [P, 128], mybir.dt.float32, kind="ExternalOutput").ap()

    with tile.TileContext(nc) as tc:
        build_body(tc, {"conf": conf}, {"x": x, "t": t}, n_reps=n_reps)
    nc.compile()
    _NC_CACHE[n_reps] = nc
    return nc


# Moment basis: T-lane j holds mu_j(t); V[j, d] = mu_j(d) for class d.
MOM_V = np.array(
    [
        [1, 1, 1, 1],   # 1
        [0, 1, 2, 3],   # t
        [0, 1, 4, 9],   # t^2
        [1, 0, 1, 2],   # |t - 1|
    ],
    dtype=np.float64,
)


def decode_conf(conf_sum: np.ndarray) -> np.ndarray:
    """[128,128] summed PSUM dump(s) -> moment-basis matrix M' [4,4].

    M'[c, j] = sum_pix elane_c * mu_j(t), with elane = (e0, e1, e2, 1)."""
    O = conf_sum.reshape(32, 4, 32, 4)
    return O[np.arange(32), :, np.arange(32), :].sum(axis=0)


def finish(Mp: np.ndarray) -> np.float32:
    """Moment-basis M' [4,4] -> dice loss scalar (f32 math as the reference)."""
    Mp = Mp.astype(np.float64)
    # rows c<3: M[c, :] (target-class histogram within pred class c)
    M_rows = np.linalg.solve(MOM_V, Mp[:3, :].T).T  # [3, 4]
    M_rows = np.rint(M_rows)
    tgt = np.rint(np.linalg.solve(MOM_V, Mp[3, :]))  # [4]
    n_tot = Mp[3, 0]
    pred = np.empty(4)
    pred[:3] = Mp[:3, 0]
    pred[3] = n_tot - pred[:3].sum()
    inter = np.empty(4)
    inter[:3] = np.diag(M_rows[:, :3])
    inter[3] = tgt[3] - M_rows[:, 3].sum()

    inter32 = inter.astype(np.float32)
    union32 = (pred + tgt).astype(np.float32)
    eps32 = np.float32(EPS)
    dice = (np.float32(2.0) * inter32 + eps32) / (union32 + eps32)
    losses = np.float32(1.0) - dice
    return np.float32(losses.mean(dtype=np.float32))


LAST_RESULT = None


def kernel(**inputs) -> np.ndarray:
    from concourse import bass_utils

    x_full = np.asarray(inputs["input"], dtype=np.float32)
    t_full = np.asarray(inputs["target"])
    t_u8 = t_full.astype(np.uint8)

    nc = _get_nc()
    in_maps = []
    for ci in range(N_CORES):
        sl = slice(ci * B_LOC, (ci + 1) * B_LOC)
        in_maps.append(
            {
                "x": np.ascontiguousarray(x_full[sl]),
                "t": np.ascontiguousarray(t_u8[sl]),
            }
        )

    # Transient NRT device errors (e.g. NRT_EXEC_UNIT_UNRECOVERABLE) have
    # been observed to succeed on retry in this environment.
    last_exc = None
    for attempt in range(3):
        try:
            res = bass_utils.run_bass_kernel_spmd(
                nc, in_maps, core_ids=list(range(N_CORES))
            )
            break
        except Exception as exc:  # noqa: BLE001
            last_exc = exc
            import time as _time

            _time.sleep(2.0 * (attempt + 1))
    else:
        raise last_exc
    global LAST_RESULT
    LAST_RESULT = res

    conf_sum = np.zeros((P, 128), dtype=np.float64)
    for r in res.results:
        conf_sum += r["conf"].astype(np.float64)
    Mp = decode_conf(conf_sum)
    return finish(Mp)


# revision 40
# speedup vs baseline: 1.0133x; 1.0133x over previous
"""DiceLoss kernel for Trainium2, data-parallel over 8 NeuronCores.

Algorithm (per core, 2 of 16 batches):
  - argmax one-hot lanes E = (e0, e1, e2, 1) with e_c = [x_c == max_c(x)],
    computed exactly: e0/e1 as f32 is_equal on the Vector engine, e2 via
    d2 = x2 - mx on Pool (exactly 0 iff x2 == mx) then d2 == 0.
  - target MOMENT lanes T = (1, t, t^2, |t-1|) straight from the uint8
    labels on the Scalar engine (one ACT op per lane; values are small
    ints, exact in bf16).
  - Both lane sets are written bf16, interleaved class-minor into
    [128, 4*fd] tiles; the TensorEngine accumulates
    O += E_chunk^T @ T_chunk over 128-wide chunks in PSUM. The 4x4
    diagonal blocks of O sum to M'[c, j] = sum_pix e_c * mu_j(t).
  - Host sums the 8 per-core [128,128] PSUM dumps, inverts the 4x4 moment
    basis (exact integers) to get the confusion matrix, and finishes the
    (2i+eps)/(u+eps) division and the mean in f32 like the reference.

All sums are integer-valued f32 < 2^24, so the result matches the jax
reference bit-for-bit (up to argmax ties that are bit-equal in f32).
"""
import sys

sys.path.insert(0, "/opt/trn_rl_repo")

import numpy as np

B, C, H, W = 16, 4, 512, 512
N_CORES = 8
B_LOC = B // N_CORES          # 2 batches per core
EPS = 1e-6
P = 128                       # SBUF partitions
FD = 1024                     # max free-dim of one pixel tile
PLANE = H * W                 # 262144 pixels per (b, c) plane

# Pixel segments per core: (batch, flat_start, fd). Each covers pixels
# [start, start + 128*fd) of that batch's plane; partition k owns
# [start + k*fd, start + (k+1)*fd). Trailing segments are smaller so the
# post-last-DMA compute tail is short.
SEGS = [
    (0, 0, 512),
    (0, 65536, 1024),
    (0, 196608, 512),
    (1, 0, 1024),
    (1, 131072, 512),
    (1, 196608, 256),
    (1, 229376, 256),
]
assert sum(128 * fd for b, s, fd in SEGS) == B_LOC * PLANE
NT = len(SEGS)
NCH_TOT = sum(4 * fd // 128 for _, _, fd in SEGS)


def build_body(tc, outs, ins, n_reps=1):
    """Kernel body. ins = {"x": AP [B_LOC,C,H,W] f32, "t": AP [B_LOC,H,W] u8}
    outs = {"conf": AP [128,128] f32}. n_reps>1 repeats the whole pass
    (PSUM keeps accumulating; used for timing-by-differencing)."""
    import concourse.mybir as mybir

    nc = tc.nc
    f32 = mybir.dt.float32
    bf16 = mybir.dt.bfloat16
    AF = mybir.ActivationFunctionType
    OP = mybir.AluOpType

    x = ins["x"]
    t = ins["t"]
    conf = outs["conf"]

    xf = x.rearrange("b c h w -> b c (h w)")
    tfl = t.rearrange("b h w -> b (h w)")

    NEB = 3  # E/T buffer count
    with (
        tc.tile_pool(name="xin", bufs=4) as xin,
        tc.tile_pool(name="work", bufs=3) as work,
        tc.tile_pool(name="eht", bufs=1) as eht,
        tc.tile_pool(name="psum", bufs=1, space="PSUM") as psum,
    ):
        P_acc = psum.tile([P, 128], f32, name="P_acc")
        bias_m1 = eht.tile([P, 1], f32, name="bias_m1")
        nc.gpsimd.memset(bias_m1, -1.0)
        Es = [eht.tile([P, FD * 4], bf16, name=f"Ebuf{i}") for i in range(NEB)]
        Ts = [eht.tile([P, FD * 4], bf16, name=f"Tbuf{i}") for i in range(NEB)]
        for buf in Es:
            b4 = buf.rearrange("p (f c) -> p f c", c=4)
            nc.gpsimd.memset(b4[:, :, 3], 1.0)
        for buf in Ts:
            b4 = buf.rearrange("p (f c) -> p f c", c=4)
            nc.gpsimd.memset(b4[:, :, 0], 1.0)

        n_mm = n_reps * NCH_TOT
        mm = 0
        for it_g in range(n_reps * NT):
            it = it_g % NT
            b_i, seg_start, fd = SEGS[it]
            npix = P * fd

            xts = [None] * C
            tu = None

            def dma_x(c):
                xc = xin.tile([P, FD], f32, name=f"xc{c}")[:, :fd]
                nc.sync.dma_start(
                    out=xc,
                    in_=xf[b_i, c, seg_start : seg_start + npix].rearrange(
                        "(p f) -> p f", f=fd
                    ),
                )
                xts[c] = xc

            dma_x(0)
            dma_x(1)
            dma_x(2)
            dma_x(3)
            tu = xin.tile([P, FD], mybir.dt.uint8, name="tu")[:, :fd]
            nc.sync.dma_start(
                out=tu,
                in_=tfl[b_i, seg_start : seg_start + npix].rearrange(
                    "(p f) -> p f", f=fd
                ),
            )

            # max over the 4 class planes (Pool has no TensorTensor max)
            m01 = work.tile([P, FD], f32, name="m01")[:, :fd]
            m23 = work.tile([P, FD], f32, name="m23")[:, :fd]
            mx = work.tile([P, FD], f32, name="mx")[:, :fd]
            nc.vector.tensor_tensor(m01, xts[0], xts[1], OP.max)
            nc.vector.tensor_tensor(m23, xts[2], xts[3], OP.max)
            nc.vector.tensor_tensor(mx, m01, m23, OP.max)

            E = Es[it_g % NEB]
            T = Ts[it_g % NEB]
            E4 = E[:, : 4 * fd].rearrange("p (f c) -> p f c", c=4)
            T4 = T[:, : 4 * fd].rearrange("p (f c) -> p f c", c=4)

            # pred one-hot lanes 0..2 (lane 3 stays 1.0):
            #   e0, e1 on DVE via is_equal(x_c, mx)
            #   e2 on Pool via d2 = x2 - mx (exactly 0 iff x2 == mx), then
            #   d2 == 0. On the last segment e2 runs on DVE: the Pool chain
            #   would sit on the critical tail.
            nc.vector.tensor_tensor(E4[:, :, 0], xts[0], mx, OP.is_equal)
            nc.vector.tensor_tensor(E4[:, :, 1], xts[1], mx, OP.is_equal)
            if it_g == n_reps * NT - 1:
                nc.vector.tensor_tensor(E4[:, :, 2], xts[2], mx, OP.is_equal)
            else:
                d2 = work.tile([P, FD], f32, name="d2")[:, :fd]
                nc.gpsimd.tensor_tensor(d2, xts[2], mx, OP.subtract)
                nc.gpsimd.tensor_scalar(E4[:, :, 2], d2, 0.0, None, OP.is_equal)

            # target MOMENT lanes on ACT, straight from the uint8 labels:
            #   lane 0 = 1 (memset), lane 1 = t, lane 2 = t^2, lane 3 = |t-1|
            # All values are small ints -> exact in bf16; the host inverts the
            # 4x4 moment basis to recover per-class counts.
            nc.scalar.copy(T4[:, :, 1], tu)
            nc.scalar.activation(T4[:, :, 2], tu, AF.Square)
            nc.scalar.activation(T4[:, :, 3], tu, AF.Abs, bias=bias_m1, scale=1.0)

            for w_i in range(4 * fd // 128):
                sl = slice(w_i * 128, (w_i + 1) * 128)
                nc.tensor.matmul(
                    P_acc,
                    E[:, sl],
                    T[:, sl],
                    start=(mm == 0),
                    stop=(mm == n_mm - 1),
                )
                mm += 1

        conf_sb = eht.tile([P, 128], f32, name="conf_sb")
        nc.scalar.copy(conf_sb, P_acc)
        nc.sync.dma_start(out=conf, in_=conf_sb)


_NC_CACHE = {}


def _get_nc(n_reps=1):
    if n_reps in _NC_CACHE:
        return _NC_CACHE[n_reps]
    import concourse.bacc as bacc
    import concourse.mybir as mybir
    import concourse.tile as tile

    nc = bacc.Bacc(
        "TRN2",
        target_bir_lowering=False,
        debug=False,
        enable_asserts=False,
        num_devices=N_CORES,
    )
    x = nc.dram_tensor("x", [B_LOC, C, H, W], mybir.dt.float32, kind="ExternalInput").ap()
    t = nc.dram_tensor("t", [B_LOC, H, W], mybir.dt.uint8, kind="ExternalInput").ap()
    conf = nc.dram_tensor("conf", [P, 128], mybir.dt.float32, kind="ExternalOutput").ap()

    with tile.TileContext(nc) as tc:
        build_body(tc, {"conf": conf}, {"x": x, "t": t}, n_reps=n_reps)
    nc.compile()
    _NC_CACHE[n_reps] = nc
    return nc


# Moment basis: T-lane j holds mu_j(t); V[j, d] = mu_j(d) for class d.
MOM_V = np.array(
    [
        [1, 1, 1, 1],   # 1
        [0, 1, 2, 3],   # t
        [0, 1, 4, 9],   # t^2
        [1, 0, 1, 2],   # |t - 1|
    ],
    dtype=np.float64,
)


def decode_conf(conf_sum: np.ndarray) -> np.ndarray:
    """[128,128] summed PSUM dump(s) -> moment-basis matrix M' [4,4].

    M'[c, j] = sum_pix elane_c * mu_j(t), with elane = (e0, e1, e2, 1)."""
    O = conf_sum.reshape(32, 4, 32, 4)
    return O[np.arange(32), :, np.arange(32), :].sum(axis=0)


def finish(Mp: np.ndarray) -> np.float32:
    """Moment-basis M' [4,4] -> dice loss scalar (f32 math as the reference)."""
    Mp = Mp.astype(np.float64)
    # rows c<3: M[c, :] (target-class histogram within pred class c)
    M_rows = np.linalg.solve(MOM_V, Mp[:3, :].T).T  # [3, 4]
    M_rows = np.rint(M_rows)
    tgt = np.rint(np.linalg.solve(MOM_V, Mp[3, :]))  # [4]
    n_tot = Mp[3, 0]
    pred = np.empty(4)
    pred[:3] = Mp[:3, 0]
    pred[3] = n_tot - pred[:3].sum()
    inter = np.empty(4)
    inter[:3] = np.diag(M_rows[:, :3])
    inter[3] = tgt[3] - M_rows[:, 3].sum()

    inter32 = inter.astype(np.float32)
    union32 = (pred + tgt).astype(np.float32)
    eps32 = np.float32(EPS)
    dice = (np.float32(2.0) * inter32 + eps32) / (union32 + eps32)
    losses = np.float32(1.0) - dice
    return np.float32(losses.mean(dtype=np.float32))


LAST_RESULT = None


def kernel(**inputs) -> np.ndarray:
    from concourse import bass_utils

    x_full = np.asarray(inputs["input"], dtype=np.float32)
    t_full = np.asarray(inputs["target"])
    t_u8 = t_full.astype(np.uint8)

    nc = _get_nc()
    in_maps = []
    for ci in range(N_CORES):
        sl = slice(ci * B_LOC, (ci + 1) * B_LOC)
        in_maps.append(
            {
                "x": np.ascontiguousarray(x_full[sl]),
                "t": np.ascontiguousarray(t_u8[sl]),
            }
        )

    # Transient NRT device errors (e.g. NRT_EXEC_UNIT_UNRECOVERABLE) have
    # been observed to succeed on retry in this environment.
    last_exc = None
    for attempt in range(3):
        try:
            res = bass_utils.run_bass_kernel_spmd(
                nc, in_maps, core_ids=list(range(N_CORES))
            )
            break
        except Exception as exc:  # noqa: BLE001
            last_exc = exc
            import time as _time

            _time.sleep(2.0 * (attempt + 1))
    else:
        raise last_exc
    global LAST_RESULT
    LAST_RESULT = res

    conf_sum = np.zeros((P, 128), dtype=np.float64)
    for r in res.results:
        conf_sum += r["conf"].astype(np.float64)
    Mp = decode_conf(conf_sum)
    return finish(Mp)


# revision 52
# speedup vs baseline: 1.0358x; 1.0222x over previous
"""DiceLoss kernel for Trainium2, data-parallel over 8 NeuronCores.

Algorithm (per core, 2 of 16 batches):
  - argmax one-hot lanes E = (e0, e1, e2, 1) with e_c = [x_c == max_c(x)],
    computed exactly: e0/e1 as f32 is_equal on the Vector engine, e2 via
    d2 = x2 - mx on Pool (exactly 0 iff x2 == mx) then d2 == 0.
  - target MOMENT lanes T = (1, t, t^2, |t-1|) straight from the uint8
    labels on the Scalar engine (one ACT op per lane; values are small
    ints, exact in bf16).
  - Both lane sets are written bf16, interleaved class-minor into
    [128, 4*fd] tiles; the TensorEngine accumulates
    O += E_chunk^T @ T_chunk over 128-wide chunks in PSUM. The 4x4
    diagonal blocks of O sum to M'[c, j] = sum_pix e_c * mu_j(t).
  - Host sums the 8 per-core [128,128] PSUM dumps, inverts the 4x4 moment
    basis (exact integers) to get the confusion matrix, and finishes the
    (2i+eps)/(u+eps) division and the mean in f32 like the reference.

All sums are integer-valued f32 < 2^24, so the result matches the jax
reference bit-for-bit (up to argmax ties that are bit-equal in f32).
"""
import sys

sys.path.insert(0, "/opt/trn_rl_repo")

import numpy as np

B, C, H, W = 16, 4, 512, 512
N_CORES = 8
B_LOC = B // N_CORES          # 2 batches per core
EPS = 1e-6
P = 128                       # SBUF partitions
FD = 1024                     # max free-dim of one pixel tile
PLANE = H * W                 # 262144 pixels per (b, c) plane

# Pixel segments per core: (batch, flat_start, fd). Each covers pixels
# [start, start + 128*fd) of that batch's plane; partition k owns
# [start + k*fd, start + (k+1)*fd). Trailing segments are smaller so the
# post-last-DMA compute tail is short.
SEGS = [
    (0, 0, 512),
    (0, 65536, 1024),
    (0, 196608, 512),
    (1, 0, 1024),
    (1, 131072, 512),
    (1, 196608, 256),
    (1, 229376, 256),
]
assert sum(128 * fd for b, s, fd in SEGS) == B_LOC * PLANE
NT = len(SEGS)
NCH_TOT = sum(4 * fd // 128 for _, _, fd in SEGS)


def build_body(tc, outs, ins, n_reps=1):
    """Kernel body. ins = {"x": AP [B_LOC,C,H,W] f32, "t": AP [B_LOC,H,W] u8}
    outs = {"conf": AP [128,128] f32}. n_reps>1 repeats the whole pass
    (PSUM keeps accumulating; used for timing-by-differencing)."""
    import concourse.mybir as mybir

    nc = tc.nc
    f32 = mybir.dt.float32
    bf16 = mybir.dt.bfloat16
    AF = mybir.ActivationFunctionType
    OP = mybir.AluOpType

    x = ins["x"]
    t = ins["t"]
    conf = outs["conf"]

    xf = x.rearrange("b c h w -> b c (h w)")
    tfl = t.rearrange("b h w -> b (h w)")

    NEB = 3  # E/T buffer count
    with (
        tc.tile_pool(name="xin", bufs=4) as xin,
        tc.tile_pool(name="work", bufs=3) as work,
        tc.tile_pool(name="eht", bufs=1) as eht,
        tc.tile_pool(name="psum", bufs=1, space="PSUM") as psum,
    ):
        P_acc = psum.tile([P, 128], f32, name="P_acc")
        bias_m1 = eht.tile([P, 1], f32, name="bias_m1")
        nc.gpsimd.memset(bias_m1, -1.0)
        Es = [eht.tile([P, FD * 4], bf16, name=f"Ebuf{i}") for i in range(NEB)]
        Ts = [eht.tile([P, FD * 4], bf16, name=f"Tbuf{i}") for i in range(NEB)]
        for buf in Es:
            b4 = buf.rearrange("p (f c) -> p f c", c=4)
            nc.gpsimd.memset(b4[:, :, 3], 1.0)
        for buf in Ts:
            b4 = buf.rearrange("p (f c) -> p f c", c=4)
            nc.gpsimd.memset(b4[:, :, 0], 1.0)

        n_mm = n_reps * NCH_TOT
        mm = 0
        for it_g in range(n_reps * NT):
            it = it_g % NT
            b_i, seg_start, fd = SEGS[it]
            npix = P * fd

            xts = [None] * C
            tu = None

            def dma_x(c):
                xc = xin.tile([P, FD], f32, name=f"xc{c}")[:, :fd]
                nc.sync.dma_start(
                    out=xc,
                    in_=xf[b_i, c, seg_start : seg_start + npix].rearrange(
                        "(p f) -> p f", f=fd
                    ),
                )
                xts[c] = xc

            dma_x(0)
            dma_x(1)
            dma_x(2)
            dma_x(3)
            tu = xin.tile([P, FD], mybir.dt.uint8, name="tu")[:, :fd]
            nc.sync.dma_start(
                out=tu,
                in_=tfl[b_i, seg_start : seg_start + npix].rearrange(
                    "(p f) -> p f", f=fd
                ),
            )

            # max over the 4 class planes (Pool has no TensorTensor max)
            m01 = work.tile([P, FD], f32, name="m01")[:, :fd]
            m23 = work.tile([P, FD], f32, name="m23")[:, :fd]
            mx = work.tile([P, FD], f32, name="mx")[:, :fd]
            nc.vector.tensor_tensor(m01, xts[0], xts[1], OP.max)
            nc.vector.tensor_tensor(m23, xts[2], xts[3], OP.max)
            nc.vector.tensor_tensor(mx, m01, m23, OP.max)

            E = Es[it_g % NEB]
            T = Ts[it_g % NEB]
            E4 = E[:, : 4 * fd].rearrange("p (f c) -> p f c", c=4)
            T4 = T[:, : 4 * fd].rearrange("p (f c) -> p f c", c=4)

            # pred one-hot lanes 0..2 (lane 3 stays 1.0):
            #   e0, e1 on DVE via is_equal(x_c, mx)
            #   e2 on Pool via d2 = x2 - mx (exactly 0 iff x2 == mx), then
            #   d2 == 0. On the last segment e2 runs on DVE: the Pool chain
            #   would sit on the critical tail.
            nc.vector.tensor_tensor(E4[:, :, 0], xts[0], mx, OP.is_equal)
            nc.vector.tensor_tensor(E4[:, :, 1], xts[1], mx, OP.is_equal)
            if it_g == n_reps * NT - 1:
                nc.vector.tensor_tensor(E4[:, :, 2], xts[2], mx, OP.is_equal)
            else:
                d2 = work.tile([P, FD], f32, name="d2")[:, :fd]
                nc.gpsimd.tensor_tensor(d2, xts[2], mx, OP.subtract)
                nc.gpsimd.tensor_scalar(E4[:, :, 2], d2, 0.0, None, OP.is_equal)

            # target MOMENT lanes on ACT, straight from the uint8 labels:
            #   lane 0 = 1 (memset), lane 1 = t, lane 2 = t^2, lane 3 = |t-1|
            # All values are small ints -> exact in bf16; the host inverts the
            # 4x4 moment basis to recover per-class counts.
            nc.scalar.copy(T4[:, :, 1], tu)
            nc.scalar.activation(T4[:, :, 2], tu, AF.Square)
            nc.scalar.activation(T4[:, :, 3], tu, AF.Abs, bias=bias_m1, scale=1.0)

            for w_i in range(4 * fd // 128):
                sl = slice(w_i * 128, (w_i + 1) * 128)
                nc.tensor.matmul(
                    P_acc,
                    E[:, sl],
                    T[:, sl],
                    start=(mm == 0),
                    stop=(mm == n_mm - 1),
                )
                mm += 1

        conf_sb = eht.tile([P, 128], f32, name="conf_sb")
        nc.scalar.copy(conf_sb, P_acc)
        nc.sync.dma_start(out=conf, in_=conf_sb)


_NC_CACHE = {}


def _get_nc(n_reps=1):
    if n_reps in _NC_CACHE:
        return _NC_CACHE[n_reps]
    import concourse.bacc as bacc
    import concourse.mybir as mybir
    import concourse.tile as tile

    nc = bacc.Bacc(
        "TRN2",
        target_bir_lowering=False,
        debug=False,
        enable_asserts=False,
        num_devices=N_CORES,
    )
    x = nc.dram_tensor("x", [B_LOC, C, H, W], mybir.dt.float32, kind="ExternalInput").ap()
    t = nc.dram_tensor("t", [B_LOC, H, W], mybir.dt.uint8, kind="ExternalInput").ap()
    conf = nc.dram_tensor("conf", [P, 128], mybir.dt.float32, kind="ExternalOutput").ap()

    with tile.TileContext(nc) as tc:
        build_body(tc, {"conf": conf}, {"x": x, "t": t}, n_reps=n_reps)
    nc.compile()
    _NC_CACHE[n_reps] = nc
    return nc


# Moment basis: T-lane j holds mu_j(t); V[j, d] = mu_j(d) for class d.
MOM_V = np.array(
    [
        [1, 1, 1, 1],   # 1
        [0, 1, 2, 3],   # t
        [0, 1, 4, 9],   # t^2
        [1, 0, 1, 2],   # |t - 1|
    ],
    dtype=np.float64,
)


def decode_conf(conf_sum: np.ndarray) -> np.ndarray:
    """[128,128] summed PSUM dump(s) -> moment-basis matrix M' [4,4].

    M'[c, j] = sum_pix elane_c * mu_j(t), with elane = (e0, e1, e2, 1)."""
    O = conf_sum.reshape(32, 4, 32, 4)
    return O[np.arange(32), :, np.arange(32), :].sum(axis=0)


def finish(Mp: np.ndarray) -> np.float32:
    """Moment-basis M' [4,4] -> dice loss scalar (f32 math as the reference)."""
    Mp = Mp.astype(np.float64)
    # rows c<3: M[c, :] (target-class histogram within pred class c)
    M_rows = np.linalg.solve(MOM_V, Mp[:3, :].T).T  # [3, 4]
    M_rows = np.rint(M_rows)
    tgt = np.rint(np.linalg.solve(MOM_V, Mp[3, :]))  # [4]
    n_tot = Mp[3, 0]
    pred = np.empty(4)
    pred[:3] = Mp[:3, 0]
    pred[3] = n_tot - pred[:3].sum()
    inter = np.empty(4)
    inter[:3] = np.diag(M_rows[:, :3])
    inter[3] = tgt[3] - M_rows[:, 3].sum()

    inter32 = inter.astype(np.float32)
    union32 = (pred + tgt).astype(np.float32)
    eps32 = np.float32(EPS)
    dice = (np.float32(2.0) * inter32 + eps32) / (union32 + eps32)
    losses = np.float32(1.0) - dice
    return np.float32(losses.mean(dtype=np.float32))


LAST_RESULT = None


def kernel(**inputs) -> np.ndarray:
    from concourse import bass_utils

    x_full = np.asarray(inputs["input"], dtype=np.float32)
    t_full = np.asarray(inputs["target"])
    t_u8 = t_full.astype(np.uint8)

    nc = _get_nc()
    in_maps = []
    for ci in range(N_CORES):
        sl = slice(ci * B_LOC, (ci + 1) * B_LOC)
        in_maps.append(
            {
                "x": np.ascontiguousarray(x_full[sl]),
                "t": np.ascontiguousarray(t_u8[sl]),
            }
        )

    # Transient NRT device errors (e.g. NRT_EXEC_UNIT_UNRECOVERABLE) have
    # been observed to succeed on retry in this environment.
    last_exc = None
    for attempt in range(3):
        try:
            res = bass_utils.run_bass_kernel_spmd(
                nc, in_maps, core_ids=list(range(N_CORES))
            )
            break
        except Exception as exc:  # noqa: BLE001
            last_exc = exc
            import time as _time

            _time.sleep(2.0 * (attempt + 1))
    else:
        raise last_exc
    global LAST_RESULT
    LAST_RESULT = res

    conf_sum = np.zeros((P, 128), dtype=np.float64)
    for r in res.results:
        conf_sum += r["conf"].astype(np.float64)
    Mp = decode_conf(conf_sum)
    return finish(Mp)


# revision 65
# speedup vs baseline: 1.0369x; 1.0010x over previous
"""DiceLoss kernel for Trainium2, data-parallel over 8 NeuronCores.

Algorithm (per core, 2 of 16 batches):
  - argmax one-hot lanes E = (e0, e1, e2, 1) with e_c = [x_c == max_c(x)],
    computed exactly: e0/e1 as f32 is_equal on the Vector engine, e2 via
    d2 = x2 - mx on Pool (exactly 0 iff x2 == mx) then d2 == 0.
  - target MOMENT lanes T = (1, t, t^2, |t-1|) straight from the uint8
    labels on the Scalar engine (one ACT op per lane; values are small
    ints, exact in bf16).
  - Both lane sets are written bf16, interleaved class-minor into
    [128, 4*fd] tiles; the TensorEngine accumulates
    O += E_chunk^T @ T_chunk over 128-wide chunks in PSUM. The 4x4
    diagonal blocks of O sum to M'[c, j] = sum_pix e_c * mu_j(t).
  - Host sums the 8 per-core [128,128] PSUM dumps, inverts the 4x4 moment
    basis (exact integers) to get the confusion matrix, and finishes the
    (2i+eps)/(u+eps) division and the mean in f32 like the reference.

All sums are integer-valued f32 < 2^24, so the result matches the jax
reference bit-for-bit (up to argmax ties that are bit-equal in f32).
"""
import sys

sys.path.insert(0, "/opt/trn_rl_repo")

import numpy as np

B, C, H, W = 16, 4, 512, 512
N_CORES = 8
B_LOC = B // N_CORES          # 2 batches per core
EPS = 1e-6
P = 128                       # SBUF partitions
FD = 1024                     # max free-dim of one pixel tile
PLANE = H * W                 # 262144 pixels per (b, c) plane

# Pixel segments per core: (batch, flat_start, fd). Each covers pixels
# [start, start + 128*fd) of that batch's plane; partition k owns
# [start + k*fd, start + (k+1)*fd). Trailing segments are smaller so the
# post-last-DMA compute tail is short.
SEGS = [
    (0, 0, 512),
    (0, 65536, 1024),
    (0, 196608, 512),
    (1, 0, 1024),
    (1, 131072, 512),
    (1, 196608, 256),
    (1, 229376, 256),
]
assert sum(128 * fd for b, s, fd in SEGS) == B_LOC * PLANE
NT = len(SEGS)
NCH_TOT = sum(4 * fd // 128 for _, _, fd in SEGS)


def build_body(tc, outs, ins, n_reps=1):
    """Kernel body. ins = {"x": AP [B_LOC,C,H,W] f32, "t": AP [B_LOC,H,W] u8}
    outs = {"conf": AP [128,128] f32}. n_reps>1 repeats the whole pass
    (PSUM keeps accumulating; used for timing-by-differencing)."""
    import concourse.mybir as mybir

    nc = tc.nc
    f32 = mybir.dt.float32
    bf16 = mybir.dt.bfloat16
    AF = mybir.ActivationFunctionType
    OP = mybir.AluOpType

    x = ins["x"]
    t = ins["t"]
    conf = outs["conf"]

    xf = x.rearrange("b c h w -> b c (h w)")
    tfl = t.rearrange("b h w -> b (h w)")

    NEB = 3  # E/T buffer count
    with (
        tc.tile_pool(name="xin", bufs=4) as xin,
        tc.tile_pool(name="work", bufs=3) as work,
        tc.tile_pool(name="eht", bufs=1) as eht,
        tc.tile_pool(name="psum", bufs=1, space="PSUM") as psum,
    ):
        P_acc = psum.tile([P, 128], f32, name="P_acc")
        bias_m1 = eht.tile([P, 1], f32, name="bias_m1")
        nc.gpsimd.memset(bias_m1, -1.0)
        Es = [eht.tile([P, FD * 4], bf16, name=f"Ebuf{i}") for i in range(NEB)]
        Ts = [eht.tile([P, FD * 4], bf16, name=f"Tbuf{i}") for i in range(NEB)]
        for buf in Es:
            b4 = buf.rearrange("p (f c) -> p f c", c=4)
            nc.gpsimd.memset(b4[:, :, 3], 1.0)
        for buf in Ts:
            b4 = buf.rearrange("p (f c) -> p f c", c=4)
            nc.gpsimd.memset(b4[:, :, 0], 1.0)

        n_mm = n_reps * NCH_TOT
        mm = 0
        for it_g in range(n_reps * NT):
            it = it_g % NT
            b_i, seg_start, fd = SEGS[it]
            npix = P * fd

            xts = [None] * C
            tu = None

            def dma_x(c):
                xc = xin.tile([P, FD], f32, name=f"xc{c}")[:, :fd]
                nc.sync.dma_start(
                    out=xc,
                    in_=xf[b_i, c, seg_start : seg_start + npix].rearrange(
                        "(p f) -> p f", f=fd
                    ),
                )
                xts[c] = xc

            dma_x(0)
            dma_x(1)
            dma_x(2)
            dma_x(3)
            tu = xin.tile([P, FD], mybir.dt.uint8, name="tu")[:, :fd]
            nc.sync.dma_start(
                out=tu,
                in_=tfl[b_i, seg_start : seg_start + npix].rearrange(
                    "(p f) -> p f", f=fd
                ),
            )

            # max over the 4 class planes (Pool has no TensorTensor max)
            m01 = work.tile([P, FD], f32, name="m01")[:, :fd]
            m23 = work.tile([P, FD], f32, name="m23")[:, :fd]
            mx = work.tile([P, FD], f32, name="mx")[:, :fd]
            nc.vector.tensor_tensor(m01, xts[0], xts[1], OP.max)
            nc.vector.tensor_tensor(m23, xts[2], xts[3], OP.max)
            nc.vector.tensor_tensor(mx, m01, m23, OP.max)

            E = Es[it_g % NEB]
            T = Ts[it_g % NEB]
            E4 = E[:, : 4 * fd].rearrange("p (f c) -> p f c", c=4)
            T4 = T[:, : 4 * fd].rearrange("p (f c) -> p f c", c=4)

            # pred one-hot lanes 0..2 (lane 3 stays 1.0):
            #   e0, e1 on DVE via is_equal(x_c, mx)
            #   e2 on Pool via d2 = x2 - mx (exactly 0 iff x2 == mx), then
            #   d2 == 0 (runs concurrently with the DVE compares).
            nc.vector.tensor_tensor(E4[:, :, 0], xts[0], mx, OP.is_equal)
            nc.vector.tensor_tensor(E4[:, :, 1], xts[1], mx, OP.is_equal)
            d2 = work.tile([P, FD], f32, name="d2")[:, :fd]
            nc.gpsimd.tensor_tensor(d2, xts[2], mx, OP.subtract)
            nc.gpsimd.tensor_scalar(E4[:, :, 2], d2, 0.0, None, OP.is_equal)

            # target MOMENT lanes on ACT, straight from the uint8 labels:
            #   lane 0 = 1 (memset), lane 1 = t, lane 2 = t^2, lane 3 = |t-1|
            # All values are small ints -> exact in bf16; the host inverts the
            # 4x4 moment basis to recover per-class counts.
            nc.scalar.copy(T4[:, :, 1], tu)
            nc.scalar.activation(T4[:, :, 2], tu, AF.Square)
            nc.scalar.activation(T4[:, :, 3], tu, AF.Abs, bias=bias_m1, scale=1.0)

            for w_i in range(4 * fd // 128):
                sl = slice(w_i * 128, (w_i + 1) * 128)
                nc.tensor.matmul(
                    P_acc,
                    E[:, sl],
                    T[:, sl],
                    start=(mm == 0),
                    stop=(mm == n_mm - 1),
                )
                mm += 1

        conf_sb = eht.tile([P, 128], f32, name="conf_sb")
        nc.scalar.copy(conf_sb, P_acc)
        nc.sync.dma_start(out=conf, in_=conf_sb)


_NC_CACHE = {}


def _get_nc(n_reps=1):
    if n_reps in _NC_CACHE:
        return _NC_CACHE[n_reps]
    import concourse.bacc as bacc
    import concourse.mybir as mybir
    import concourse.tile as tile

    nc = bacc.Bacc(
        "TRN2",
        target_bir_lowering=False,
        debug=False,
        enable_asserts=False,
        num_devices=N_CORES,
    )
    x = nc.dram_tensor("x", [B_LOC, C, H, W], mybir.dt.float32, kind="ExternalInput").ap()
    t = nc.dram_tensor("t", [B_LOC, H, W], mybir.dt.uint8, kind="ExternalInput").ap()
    conf = nc.dram_tensor("conf", [P, 128], mybir.dt.float32, kind="ExternalOutput").ap()

    with tile.TileContext(nc) as tc:
        build_body(tc, {"conf": conf}, {"x": x, "t": t}, n_reps=n_reps)
    nc.compile()
    _NC_CACHE[n_reps] = nc
    return nc


# Moment basis: T-lane j holds mu_j(t); V[j, d] = mu_j(d) for class d.
MOM_V = np.array(
    [
        [1, 1, 1, 1],   # 1
        [0, 1, 2, 3],   # t
        [0, 1, 4, 9],   # t^2
        [1, 0, 1, 2],   # |t - 1|
    ],
    dtype=np.float64,
)


def decode_conf(conf_sum: np.ndarray) -> np.ndarray:
    """[128,128] summed PSUM dump(s) -> moment-basis matrix M' [4,4].

    M'[c, j] = sum_pix elane_c * mu_j(t), with elane = (e0, e1, e2, 1)."""
    O = conf_sum.reshape(32, 4, 32, 4)
    return O[np.arange(32), :, np.arange(32), :].sum(axis=0)


def finish(Mp: np.ndarray) -> np.float32:
    """Moment-basis M' [4,4] -> dice loss scalar (f32 math as the reference)."""
    Mp = Mp.astype(np.float64)
    # rows c<3: M[c, :] (target-class histogram within pred class c)
    M_rows = np.linalg.solve(MOM_V, Mp[:3, :].T).T  # [3, 4]
    M_rows = np.rint(M_rows)
    tgt = np.rint(np.linalg.solve(MOM_V, Mp[3, :]))  # [4]
    n_tot = Mp[3, 0]
    pred = np.empty(4)
    pred[:3] = Mp[:3, 0]
    pred[3] = n_tot - pred[:3].sum()
    inter = np.empty(4)
    inter[:3] = np.diag(M_rows[:, :3])
    inter[3] = tgt[3] - M_rows[:, 3].sum()

    inter32 = inter.astype(np.float32)
    union32 = (pred + tgt).astype(np.float32)
    eps32 = np.float32(EPS)
    dice = (np.float32(2.0) * inter32 + eps32) / (union32 + eps32)
    losses = np.float32(1.0) - dice
    return np.float32(losses.mean(dtype=np.float32))


LAST_RESULT = None


def kernel(**inputs) -> np.ndarray:
    from concourse import bass_utils

    x_full = np.asarray(inputs["input"], dtype=np.float32)
    t_full = np.asarray(inputs["target"])
    t_u8 = t_full.astype(np.uint8)

    nc = _get_nc()
    in_maps = []
    for ci in range(N_CORES):
        sl = slice(ci * B_LOC, (ci + 1) * B_LOC)
        in_maps.append(
            {
                "x": np.ascontiguousarray(x_full[sl]),
                "t": np.ascontiguousarray(t_u8[sl]),
            }
        )

    # Transient NRT device errors (e.g. NRT_EXEC_UNIT_UNRECOVERABLE) have
    # been observed to succeed on retry in this environment.
    last_exc = None
    for attempt in range(3):
        try:
            res = bass_utils.run_bass_kernel_spmd(
                nc, in_maps, core_ids=list(range(N_CORES))
            )
            break
        except Exception as exc:  # noqa: BLE001
            last_exc = exc
            import time as _time

            _time.sleep(2.0 * (attempt + 1))
    else:
        raise last_exc
    global LAST_RESULT
    LAST_RESULT = res

    conf_sum = np.zeros((P, 128), dtype=np.float64)
    for r in res.results:
        conf_sum += r["conf"].astype(np.float64)
    Mp = decode_conf(conf_sum)
    return finish(Mp)
